# revision 1
# baseline (speedup 1.0000x reference)
"""Trainium2 Bass kernel for nn_Absolute_attention (sparse_attention).

Reference math (b=4, l=4096, dim=1024, h=16, hd=64):
    q = softmax((x @ Wq.T).reshape(b,l,h,hd+1), -1)
    time encoding: qk_weight = (1-q[...,-1]) * sum_d(time^2)  where
        sum_d(time[l,h,:]^2) = inv_hd * sum_j((c+s)^2 + (c-s)^2) = 2 exactly,
        so qk_weight = 2*(1-q_last)  (time/cos/sin cancel analytically).
    k = softmax((x @ Wk.T).reshape(b,l,h,hd), -1) * mask
    v = x @ Wv.T
    out = ((qk_weight[...,None]*k).reshape(b,l,h*hd) * v) @ Wo.T + bo

Everything is pointwise per (b,l) row -> pure data-parallel row sharding:
16384 rows over 8 cores = 2048 rows/core, 16 blocks of 128 rows.

Per 128-row block (layout: rows on partitions):
    z = x_blk @ Wcat.T  (Wcat = [Wq;Wk;Wv], 3088 cols) via PE fp32r matmuls,
        contraction over dim in 8 chunks of 128 (stationary = x.T chunks).
    e = exp(z[:, :2064])  (q+k logits; softmax without max-subtraction --
        logits are O(+-4), exp is safe in fp32)
    denq = segmented sum e_q (16 groups of 65); denk = seg sum e_k (16x64)
    G = 2*mask*(denq - eq_last) / (denq*denk)
    a = e_k * v * G[head-broadcast]   (fp32r)
    aT = PE transpose of a (8x 128x128)
    out = aT.T @ Wo.T + bo  via PE fp32r, then DMA out.

All matmuls use fp32r (TF32-like, ~1.6e-4 rel err, 4x faster than fp32).
"""
import numpy as np

import concourse.bacc as bacc
import concourse.mybir as mybir
import concourse.tile as tile
from concourse.bass_utils import run_bass_kernel_spmd

FP32 = mybir.dt.float32
F32R = mybir.dt.float32r
AX = mybir.AxisListType.X
ADD = mybir.AluOpType.add
EXP = mybir.ActivationFunctionType.Exp

B, L, DIM, H, HD = 4, 4096, 1024, 16, 64
ROWS = B * L                      # 16384
NCORES = 8
CROWS = ROWS // NCORES            # 2048
NBLK = CROWS // 128               # 16
NQ = H * (HD + 1)                 # 1040
NK = H * HD                       # 1024
QK = NQ + NK                      # 2064
TOT = QK + NK                     # 3088 (q | k | v)
NDC = DIM // 128                  # 8 contraction chunks

# N-chunks of the projection output; each fits one PSUM bank.
# First 5 cover the exp region [0, 2064), last 2 cover v.
CHUNKS = [(0, 512), (512, 512), (1024, 512), (1536, 512), (2048, 16),
          (2064, 512), (2576, 512)]
WTBASE = []
_acc = 0
for _off, _sz in CHUNKS:
    WTBASE.append(_acc)
    _acc += NDC * _sz
WTCOLS = _acc                     # 24704

_CACHE = {}


def _build():
    nc = bacc.Bacc("TRN2", target_bir_lowering=False, debug=False)
    xt_d = nc.dram_tensor("xt", [NBLK, 128, 1024], F32R, kind="ExternalInput").ap()
    wt_d = nc.dram_tensor("wt", [128, WTCOLS], F32R, kind="ExternalInput").ap()
    wo_d = nc.dram_tensor("wo", [128, NDC * 1024], F32R, kind="ExternalInput").ap()
    bo_d = nc.dram_tensor("bo", [128, 1024], FP32, kind="ExternalInput").ap()
    m_d = nc.dram_tensor("msk", [128, NBLK], FP32, kind="ExternalInput").ap()
    id_d = nc.dram_tensor("ident", [128, 128], F32R, kind="ExternalInput").ap()
    out_d = nc.dram_tensor("out", [NBLK, 128, 1024], FP32, kind="ExternalOutput").ap()

    with tile.TileContext(nc) as tc:
        with (
            tc.tile_pool(name="const", bufs=1) as cp,
            tc.tile_pool(name="xp", bufs=3) as xp,
            tc.tile_pool(name="ep", bufs=2) as ep,
            tc.tile_pool(name="t1p", bufs=2) as t1p,
            tc.tile_pool(name="ap_", bufs=2) as apool,
            tc.tile_pool(name="atp", bufs=2) as atp,
            tc.tile_pool(name="op", bufs=2) as op,
            tc.tile_pool(name="sp", bufs=2) as sp,
            tc.tile_pool(name="pp", bufs=4, space="PSUM") as pp,
            tc.tile_pool(name="tp", bufs=2, space="PSUM") as tp,
            tc.tile_pool(name="outp", bufs=2, space="PSUM") as outp,
        ):
            wt = cp.tile([128, WTCOLS], F32R, tag="wt")
            wo = cp.tile([128, NDC * 1024], F32R, tag="wo")
            bo = cp.tile([128, 1024], FP32, tag="bo")
            msk = cp.tile([128, NBLK], FP32, tag="msk")
            ident = cp.tile([128, 128], F32R, tag="ident")
            # per-chunk weight loads so the first matmuls don't wait on all 12.6MB
            for k, (off, sz) in enumerate(CHUNKS):
                lo, hi = WTBASE[k], WTBASE[k] + NDC * sz
                nc.sync.dma_start(wt[:, lo:hi], wt_d[:, lo:hi])
            nc.sync.dma_start(msk[:], m_d[:])
            nc.sync.dma_start(ident[:], id_d[:])
            nc.sync.dma_start(wo[:], wo_d[:])
            nc.sync.dma_start(bo[:], bo_d[:])

            def proj_chunk(xt, k):
                """Accumulate projection chunk k into a psum tile."""
                off, sz = CHUNKS[k]
                ps = pp.tile([128, sz], FP32, tag="pp")
                for c in range(NDC):
                    nc.tensor.matmul(
                        ps[:], xt[:, c * 128:(c + 1) * 128],
                        wt[:, WTBASE[k] + c * sz: WTBASE[k] + (c + 1) * sz],
                        start=(c == 0), stop=(c == NDC - 1))
                return ps

            def tail(st):
                """Transpose a -> aT, final matmul + bias, DMA out (for a
                finished block)."""
                i, a = st
                at = atp.tile([128, 1024], F32R, tag="at")
                for c in range(NDC):
                    tps = tp.tile([128, 128], F32R, tag="tp")
                    nc.tensor.transpose(tps[:], a[:, c * 128:(c + 1) * 128],
                                        ident[:])
                    nc.scalar.copy(at[:, c * 128:(c + 1) * 128], tps[:])
                outsb = op.tile([128, 1024], FP32, tag="outsb")
                for half in range(2):
                    ops = outp.tile([128, 512], FP32, tag="outp")
                    for c in range(NDC):
                        nc.tensor.matmul(
                            ops[:], at[:, c * 128:(c + 1) * 128],
                            wo[:, c * 1024 + half * 512: c * 1024 + half * 512 + 512],
                            start=(c == 0), stop=(c == NDC - 1))
                    nc.vector.tensor_add(outsb[:, half * 512:(half + 1) * 512],
                                         ops[:], bo[:, half * 512:(half + 1) * 512])
                nc.sync.dma_start(out_d[i], outsb[:])

            prev = None
            for i in range(NBLK):
                xt = xp.tile([128, 1024], F32R, tag="xt")
                nc.sync.dma_start(xt[:], xt_d[i])

                e = ep.tile([128, QK], FP32, tag="e")
                # exp chunks 0-3 (512-wide)
                for k in range(4):
                    off, sz = CHUNKS[k]
                    ps = proj_chunk(xt, k)
                    nc.scalar.activation(e[:, off:off + sz], ps[:], EXP)
                # chunk 4 (16-wide) interleaved with chunk 5 to hide its
                # per-matmul weight loads under 512-wide streams
                off4, sz4 = CHUNKS[4]
                off5, sz5 = CHUNKS[5]
                ps4 = pp.tile([128, sz4], FP32, tag="pp")
                ps5 = pp.tile([128, sz5], FP32, tag="pp")
                for c in range(NDC):
                    nc.tensor.matmul(
                        ps5[:], xt[:, c * 128:(c + 1) * 128],
                        wt[:, WTBASE[5] + c * sz5: WTBASE[5] + (c + 1) * sz5],
                        start=(c == 0), stop=(c == NDC - 1))
                    nc.tensor.matmul(
                        ps4[:], xt[:, c * 128:(c + 1) * 128],
                        wt[:, WTBASE[4] + c * sz4: WTBASE[4] + (c + 1) * sz4],
                        start=(c == 0), stop=(c == NDC - 1))
                nc.scalar.activation(e[:, off4:off4 + sz4], ps4[:], EXP)
                ps6 = proj_chunk(xt, 6)

                # softmax statistics and gate
                eq = e[:, 0:NQ].rearrange("p (h j) -> p h j", j=HD + 1)
                ek = e[:, NQ:QK].rearrange("p (h j) -> p h j", j=HD)
                denq = sp.tile([128, H], FP32, tag="denq")
                denk = sp.tile([128, H], FP32, tag="denk")
                eql = sp.tile([128, H], FP32, tag="eql")
                g = sp.tile([128, H], FP32, tag="g")
                nc.vector.tensor_reduce(denq[:], eq, axis=AX, op=ADD)
                nc.vector.tensor_reduce(denk[:], ek, axis=AX, op=ADD)
                nc.vector.tensor_copy(eql[:], eq[:, :, HD])
                nc.vector.tensor_sub(g[:], denq[:], eql[:])        # denq-eqlast
                nc.vector.tensor_mul(denq[:], denq[:], denk[:])    # denq*denk
                nc.vector.reciprocal(denk[:], denq[:])             # 1/(dq*dk)
                nc.vector.tensor_mul(g[:], g[:], denk[:])
                # msk holds 2*attention_mask -> G = 2*mask*(dq-el)/(dq*dk)
                nc.vector.tensor_scalar_mul(g[:], g[:], msk[:, i:i + 1])

                # a = e_k * v * G  (v chunks are still in PSUM)
                t1 = t1p.tile([128, 1024], FP32, tag="t1")
                nc.vector.tensor_mul(t1[:, 0:512], e[:, QK - 1024:QK - 512], ps5[:])
                nc.vector.tensor_mul(t1[:, 512:1024], e[:, QK - 512:QK], ps6[:])
                a = apool.tile([128, 1024], F32R, tag="a")
                nc.vector.tensor_mul(
                    a[:].rearrange("p (h j) -> p h j", j=HD),
                    t1[:].rearrange("p (h j) -> p h j", j=HD),
                    g[:].to_broadcast((128, H, HD)))

                if prev is not None:
                    tail(prev)
                prev = (i, a)
            tail(prev)
    nc.compile()
    return nc


def _host_prep(x, attention_mask, Wq, Wk, Wv, Wo, bo):
    x_flat = np.ascontiguousarray(np.asarray(x, dtype=np.float32)).reshape(ROWS, DIM)
    Wcat_T = np.ascontiguousarray(
        np.concatenate([np.asarray(Wq, np.float32), np.asarray(Wk, np.float32),
                        np.asarray(Wv, np.float32)], axis=0).T)  # [1024, 3088]
    cols = []
    for off, sz in CHUNKS:
        for c in range(NDC):
            cols.append(Wcat_T[c * 128:(c + 1) * 128, off:off + sz])
    wt_host = np.ascontiguousarray(np.concatenate(cols, axis=1))  # [128, 24704]

    wo_host = np.ascontiguousarray(
        np.asarray(Wo, np.float32).T.reshape(NDC, 128, 1024)
        .transpose(1, 0, 2).reshape(128, NDC * 1024))
    bo_host = np.ascontiguousarray(
        np.broadcast_to(np.asarray(bo, np.float32), (128, 1024)))
    id_host = np.eye(128, dtype=np.float32)
    m_flat = (2.0 * np.asarray(attention_mask, np.float32)).reshape(ROWS)

    in_maps = []
    for i in range(NCORES):
        sl = slice(i * CROWS, (i + 1) * CROWS)
        xt = np.ascontiguousarray(
            x_flat[sl].reshape(NBLK, 128, NDC, 128).transpose(0, 3, 2, 1)
        ).reshape(NBLK, 128, 1024)
        mc = np.ascontiguousarray(m_flat[sl].reshape(NBLK, 128).T)
        in_maps.append({"xt": xt, "wt": wt_host, "wo": wo_host,
                        "bo": bo_host, "msk": mc, "ident": id_host})
    return in_maps


def run(inputs, trace=False):
    """Run the kernel; returns (output, exec_time_ns or None)."""
    if "nc" not in _CACHE:
        _CACHE["nc"] = _build()
    nc = _CACHE["nc"]
    in_maps = _host_prep(
        inputs["x"], inputs["attention_mask"], inputs["Wq"], inputs["Wk"],
        inputs["Wv"], inputs["Wo"], inputs["bo"])
    res = run_bass_kernel_spmd(nc, in_maps, list(range(NCORES)), trace=trace)
    out = np.concatenate(
        [res.results[i]["out"].reshape(CROWS, DIM) for i in range(NCORES)],
        axis=0).reshape(B, L, DIM)
    return out, res.exec_time_ns


def kernel(**inputs) -> np.ndarray:
    assert inputs["x"].shape == (B, L, DIM)
    out, _ = run(inputs, trace=False)
    return out
